# revision 13
# baseline (speedup 1.0000x reference)
"""Trainium2 Bass kernel for causal RBF (squared-exponential) attention.

  p_ij = exp(-sm * ||q_i - k_j||^2) causal-masked, out = p @ v (no normalization)
  B,H,S,D = 2,16,2048,64 ; sm = 0.125

Sharding: B*H = 32 heads, 4 heads per core across 8 NeuronCores (head
parallel, no cross-core comm).

Factorization: p = D_q . exp(2*sm*qk - sm*||k||^2) with D_q diagonal in
exp(-sm*||q||^2). D_q is applied to the output rows on the host (O(N));
the device computes, per 128-key x 512-query tile, one fp16 matmul with a
66-deep contraction (rows 0..63 = K^T/Q^T, rows 64/65 = ksq split hi/lo
against -1/2 const rows), then ScalarE evaluates
    pt = exp(0.25 * psum + C)       (psum = qk - ksq/2)
straight PSUM -> SBUF. C is a host-supplied bias chosen from max(qsq) so
that pt <= e^{0.125 qsq + C} stays inside fp16 range (p <= 1 identity).

v2 scheduling insights (measured on this part):
  * The PE HAM clock gate only releases (1.2 -> 2.4 GHz) when the PE
    issue stream is gap-free for ~3.4us, and re-throttles on micro-idles.
  * LDWEIGHTS of consecutive matmuls overlap ONLY when the stationary
    free size (M = output partition width) is unchanged; M flips
    serialize LDW (+~112ns per MM).
  So v2 makes EVERY matmul M=128: V is zero-padded to [S,128] so the PV
  stationary is [128,128] (po rows 64..127 accumulate zeros); the causal
  mask is applied by an identity-stationary matmul accumulating a -60000
  fp16 bias tile into the diagonal corner (PE, not DVE); and the PE
  stream is kept dense with zero-stationary filler matmuls that
  accumulate exact zeros into the open PV bank (no extra PSUM, no deps).
PV: out^T[128, 512] += Vpad^T_kk @ P^T_kk accumulated in PSUM over kk;
rows 0..63 are the real output. out^T goes back in [D, S] layout; the
host transposes and applies D_q * e^{-C}. All O(S^2) work stays on
device.

Schedule notes: logits PSUM pool is GROUP=2 tiles x LG_BUFS=3 buffers
(6 banks) + 2 PV banks = 8. Output DMAs ride the gpsimd SWDGE queue so
the SP HWDGE queue never head-of-line blocks the next head's input
DMAs. Input DMAs are split per span so the first matmul starts after
~1/4 of the first transfer. A 1-element exp at t=0 pulls the ACT table
load off the critical path.
"""

import os
import sys

if "/opt/trn_rl_repo" not in sys.path:
    sys.path.insert(0, "/opt/trn_rl_repo")

import numpy as np

B, H, S, D = 2, 16, 2048, 64
SM = 0.125
N_CORES = 8
HPC = (B * H) // N_CORES  # heads per core = 4
SPAN = 512  # query-span per PSUM accumulation group
NSPAN = S // SPAN  # 4
KTILE = 128  # key rows per logits tile
EXP_SCALE = 2.0 * SM  # 0.25
MASK_NEG = -60000.0  # fp16-representable; 0.25*(-60000) -> exp underflows to 0

# knobs
GROUP = int(os.environ.get("KRN_GROUP", "2"))  # logits tiles per ACTIVATE
LG_BUFS = int(os.environ.get("KRN_LG_BUFS", "3"))  # logits psum buffers
WARMUP = int(os.environ.get("KRN_WARMUP", "8"))  # PE warmup matmuls
PT_BUFS = int(os.environ.get("KRN_PT_BUFS", "6"))
SPLIT_IN = os.environ.get("KRN_SPLIT_IN", "1") == "1"  # per-span input DMAs
ODMA_GP = os.environ.get("KRN_ODMA_GP", "1") == "1"  # out DMA on gpsimd queue
FILL_N = int(os.environ.get("KRN_FILL_N", "1"))  # zero-fill MMs per chunk
FILL_FREE = int(os.environ.get("KRN_FILL_FREE", "512"))  # zero-fill MM width
FILL_AUTO = os.environ.get("KRN_FILL_AUTO", "1") == "1"  # model-based fill size
FILL_ADJ = float(os.environ.get("KRN_FILL_ADJ", "0"))  # ns bias on fill budget
PE_MASK = os.environ.get("KRN_PE_MASK", "1") == "1"  # mask via PE (else DVE)

_CACHE = {}


def _build_module():
    """Build + compile the Bass module (once per process per variant)."""
    key = (GROUP, LG_BUFS, WARMUP, PT_BUFS, SPLIT_IN, ODMA_GP, FILL_N, FILL_FREE,
           PE_MASK)
    if key in _CACHE:
        return _CACHE[key]

    import concourse.mybir as mybir
    import concourse.tile as tile
    from concourse import bacc

    f32 = mybir.dt.float32
    mmdt = mybir.dt.float16

    nc = bacc.Bacc(
        "TRN2", target_bir_lowering=False, debug=False, num_devices=N_CORES
    )

    qT = nc.dram_tensor("qT", [HPC, D + 2, S], mmdt, kind="ExternalInput").ap()
    kT = nc.dram_tensor("kT", [HPC, D + 2, S], mmdt, kind="ExternalInput").ap()
    v = nc.dram_tensor("v", [HPC, S, 128], mmdt, kind="ExternalInput").ap()
    mbias = nc.dram_tensor("mbias", [128, 128], mmdt, kind="ExternalInput").ap()
    ident = nc.dram_tensor("ident", [128, 128], mmdt, kind="ExternalInput").ap()
    mbias32 = nc.dram_tensor("mbias32", [128, 128], f32, kind="ExternalInput").ap()
    biasc = nc.dram_tensor("biasc", [128, 1], f32, kind="ExternalInput").ap()
    ot = nc.dram_tensor("ot", [HPC, D, S], f32, kind="ExternalOutput").ap()

    out_dma = nc.gpsimd.dma_start if ODMA_GP else nc.sync.dma_start

    with tile.TileContext(nc) as tc:
        with (
            tc.tile_pool(name="consts", bufs=1) as consts,
            tc.tile_pool(name="qk_sb", bufs=2) as qk_sb,
            tc.tile_pool(name="v_sb", bufs=2) as v_sb,
            tc.tile_pool(name="pt_sb", bufs=PT_BUFS) as pt_sb,
            tc.tile_pool(name="ot_sb", bufs=2) as ot_sb,
            tc.tile_pool(name="lg_ps", bufs=LG_BUFS, space="PSUM") as lg_ps,
            tc.tile_pool(name="pv_ps", bufs=2, space="PSUM") as pv_ps,
        ):
            # tiny exp at t=0: pulls the ~2.7us ACT table load off the
            # critical path (overlaps the first input DMAs + PE warmups)
            actw = consts.tile([1, 1], f32, tag="actw")
            nc.gpsimd.memset(actw, 0.0)
            actwo = consts.tile([1, 1], f32, tag="actwo")
            nc.scalar.activation(
                actwo, actw, mybir.ActivationFunctionType.Exp, bias=0.0, scale=1.0
            )

            # consts ride the Activation HWDGE queue (idle at startup) so the
            # SP queue dispatches kta0/qta0/v0 without delay
            biassb = consts.tile([128, 1], f32, tag="bias")
            nc.scalar.dma_start(out=biassb, in_=biasc)
            if PE_MASK:
                mbsb = consts.tile([128, 128], mmdt, tag="mbias")
                nc.scalar.dma_start(out=mbsb, in_=mbias)
                idsb = consts.tile([128, 128], mmdt, tag="ident")
                nc.scalar.dma_start(out=idsb, in_=ident)
            else:
                mbsb32 = consts.tile([128, 128], f32, tag="mbias32")
                nc.scalar.dma_start(out=mbsb32, in_=mbias32)

            # warmup / filler sources: no DMA dependency
            wsrc = consts.tile([128, SPAN], mmdt, tag="wsrc")
            nc.gpsimd.memset(wsrc, 1.0)
            wzero = consts.tile([128, 128], mmdt, tag="wzero")
            nc.gpsimd.memset(wzero, 0.0)

            # dense dummy matmuls at start: trip the PE HAM clock-gate and
            # bridge into the first real matmuls (which wait on DMA).
            for w in range(WARMUP):
                wps = pv_ps.tile([128, SPAN], f32, tag="po")
                nc.tensor.matmul(
                    wps, wsrc[:, 0:128], wsrc[:, 0:SPAN], start=True, stop=True
                )

            # --- static chunk geometry (identical for every head) ---
            # Used to size the zero-fill matmuls so the PE segment between
            # consecutive exp-ready points matches each exp's duration:
            #   fill(c-1) + PV(c-1) + L(c+1) + mask(c+1)  ~=  exp_ns(c)
            # (PE stream: L_c, mask_c, fill(c-1), PV(c-1), L_{c+1}, ...)
            ndiag_ = SPAN // KTILE
            geo = []  # per chunk: dict(L_ns, mask_ns, pv_ns, exp_ns)
            for s_ in range(NSPAN):
                nkk_ = (s_ + 1) * ndiag_
                for g0 in range(0, nkk_, GROUP):
                    gkk_ = list(range(g0, min(g0 + GROUP, nkk_)))
                    n_ = len(gkk_)
                    gjd0_ = gkk_[0] - s_ * ndiag_
                    gc0_ = gjd0_ * KTILE if gjd0_ > 0 else 0
                    nmask_ = sum(1 for kk in gkk_ if kk - s_ * ndiag_ >= 0)
                    pv_cols_ = sum(
                        SPAN - (jd * KTILE if jd > 0 else 0)
                        for jd in (kk - s_ * ndiag_ for kk in gkk_)
                    )
                    geo.append(
                        dict(
                            L_ns=n_ * ((SPAN - gc0_) / 2.4 + 2.5),
                            mask_ns=nmask_ * (KTILE / 2.4 + 2.5),
                            pv_ns=pv_cols_ / 2.4 + n_ * 2.5,
                            exp_ns=n_ * (SPAN - gc0_) * 0.8333 + 261.0,
                        )
                    )
            NCH = len(geo)

            def fill_cols_for(ci):
                # pending chunk index ci is PV'd during chunk ci+1; size the
                # fill so ACT never starves at exp(ci+1) (cyclic across heads)
                if not FILL_AUTO:
                    return [FILL_FREE] * FILL_N
                budget = (
                    geo[(ci + 1) % NCH]["exp_ns"]
                    - geo[ci]["pv_ns"]
                    - geo[(ci + 2) % NCH]["L_ns"]
                    - geo[(ci + 2) % NCH]["mask_ns"]
                    + FILL_ADJ
                )
                cols = int(budget * 2.4 // 64) * 64
                out = []
                while cols >= 64:
                    c = min(cols, SPAN)
                    out.append(c)
                    cols -= c
                return out

            def emit_pv(pend, tail=False):
                # PV matmuls for a completed exp group (trails the logits of
                # the next group so the in-order PE stream never stalls on
                # ACT). Returns a deferred span-finalize closure, or None.
                po_, pt_, gkk_, nkk_, s_, vsb_, h_, ci_ = pend
                ndiag = SPAN // KTILE
                # zero-stationary fillers: accumulate exact zeros into the
                # open PV bank. They have no input deps, so the PE chews on
                # them while ACT finishes the exp this group's PV waits on —
                # keeps the issue stream gap-free (HAM stays at K=8/8).
                for fc in [] if tail else fill_cols_for(ci_):
                    nc.tensor.matmul(
                        po_[:, 0:fc],
                        wzero,
                        wsrc[:, 0:fc],
                        start=False,
                        stop=False,
                        skip_group_check=True,
                    )
                for j, kk in enumerate(gkk_):
                    jd = kk - s_ * ndiag
                    first, last = kk == 0, kk == nkk_ - 1
                    c0 = jd * KTILE if jd > 0 else 0
                    nc.tensor.matmul(
                        po_[:, c0:SPAN],
                        vsb_[:, kk, :],
                        pt_[:, j, c0:SPAN],
                        start=first,
                        stop=last,
                        skip_group_check=True,
                    )
                if gkk_[-1] != nkk_ - 1:
                    return None

                def finalize():
                    # span finished: copy PSUM->SBUF and DMA out. Emitted one
                    # chunk late so the copy never head-of-line blocks the
                    # next chunk's work on the in-order DVE queue.
                    oT = ot_sb.tile([D, SPAN], f32, tag="oT")
                    nc.vector.tensor_copy(oT, po_[0:D, :])
                    # very last transfer takes the low-latency SP HWDGE path
                    # (queue is empty by then); the rest stay off SP so they
                    # never head-of-line block the next head's input DMAs
                    last = h_ == HPC - 1 and s_ == NSPAN - 1
                    dma = nc.sync.dma_start if last else out_dma
                    dma(out=ot[h_, :, s_ * SPAN : (s_ + 1) * SPAN], in_=oT)

                return finalize

            pending = None
            pending_out = None

            for h in range(HPC):
                qta = qk_sb.tile([D + 2, S], mmdt, tag="qta")
                kta = qk_sb.tile([D + 2, S], mmdt, tag="kta")
                vsb = v_sb.tile([128, S // 128, 128], mmdt, tag="vsb")
                vr = v[h].rearrange("(t p) d -> p t d", p=128)
                if SPLIT_IN and h == 0:
                    # head 0 only: per-span pieces so span 0's gate
                    # (kta0+qta0) lands after 2 transfers; later heads
                    # prefetch behind compute so whole-tensor DMAs are
                    # cheaper on the HWDGE queue (fixed per-DMA cost)
                    for s in range(NSPAN):
                        sl = slice(s * SPAN, (s + 1) * SPAN)
                        nc.sync.dma_start(out=kta[:, sl], in_=kT[h][:, sl])
                        # span-0 qta rides the idle Pool SWDGE queue so
                        # kta0/qta0 dispatch in parallel (SP sequencer
                        # dispatch is ~650ns serial per DMA)
                        qdma = nc.gpsimd.dma_start if s == 0 else nc.sync.dma_start
                        qdma(out=qta[:, sl], in_=qT[h][:, sl])
                        tl = slice(
                            s * (S // 128 // NSPAN), (s + 1) * (S // 128 // NSPAN)
                        )
                        nc.sync.dma_start(out=vsb[:, tl, :], in_=vr[:, tl, :])
                else:
                    nc.sync.dma_start(out=qta, in_=qT[h])
                    nc.sync.dma_start(out=kta, in_=kT[h])
                    nc.sync.dma_start(out=vsb, in_=vr)

                ci = -1
                for s in range(NSPAN):
                    ndiag = SPAN // KTILE
                    nkk = (s + 1) * ndiag  # causal: key tiles 0..nkk-1
                    po = pv_ps.tile([128, SPAN], f32, tag="po")
                    qspan = qta[:, s * SPAN : (s + 1) * SPAN]
                    chunks = [
                        list(range(g0, min(g0 + GROUP, nkk)))
                        for g0 in range(0, nkk, GROUP)
                    ]
                    for gkk in chunks:
                        ci += 1
                        n = len(gkk)
                        pl = lg_ps.tile([128, GROUP, SPAN], f32, tag="pl")
                        gjd0 = gkk[0] - s * ndiag
                        gc0 = gjd0 * KTILE if gjd0 > 0 else 0
                        diag_js = [
                            (j, kk - s * ndiag)
                            for j, kk in enumerate(gkk)
                            if kk - s * ndiag >= 0
                        ]
                        for j, kk in enumerate(gkk):
                            jd = kk - s * ndiag
                            # trim only to the group-common dead prefix so
                            # the grouped exp below reads fully-written PSUM
                            nc.tensor.matmul(
                                pl[:, j, gc0:SPAN],
                                kta[:, kk * KTILE : (kk + 1) * KTILE],
                                qspan[:, gc0:SPAN],
                                start=True,
                                stop=not (PE_MASK and jd >= 0),
                            )
                        if PE_MASK:
                            # causal mask via PE: identity-stationary matmul
                            # accumulates the -60000 corner into the diagonal
                            # 128x128 block (keeps the mask off the DVE and
                            # the stationary M=128 so LDWEIGHTS stays hidden)
                            for j, jd in diag_js:
                                nc.tensor.matmul(
                                    pl[:, j, jd * KTILE : (jd + 1) * KTILE],
                                    idsb,
                                    mbsb,
                                    start=False,
                                    stop=True,
                                    skip_group_check=True,
                                )
                        else:
                            for j, jd in diag_js:
                                nc.vector.tensor_add(
                                    pl[:, j, jd * KTILE : (jd + 1) * KTILE],
                                    pl[:, j, jd * KTILE : (jd + 1) * KTILE],
                                    mbsb32,
                                )
                        if pending is not None:
                            fin = emit_pv(pending)
                            pending = None
                            if pending_out is not None:
                                pending_out()
                            pending_out = fin
                        pt = pt_sb.tile([128, GROUP, SPAN], mmdt, tag="pt")
                        nc.scalar.activation(
                            pt[:, 0:n, gc0:SPAN],
                            pl[:, 0:n, gc0:SPAN],
                            mybir.ActivationFunctionType.Exp,
                            bias=biassb,
                            scale=EXP_SCALE,
                        )
                        pending = (po, pt, gkk, nkk, s, vsb, h, ci)
            if pending is not None:
                fin = emit_pv(pending, tail=True)
                pending = None
                if pending_out is not None:
                    pending_out()
                if fin is not None:
                    fin()
                pending_out = None

    nc.compile()
    _CACHE[key] = nc
    return nc


def _host_prep(q, k, v):
    """Shard + relayout inputs for the 8 cores. Returns (in_maps, row_scale)."""
    q = np.ascontiguousarray(np.asarray(q, dtype=np.float32)).reshape(B * H, S, D)
    k = np.ascontiguousarray(np.asarray(k, dtype=np.float32)).reshape(B * H, S, D)
    v = np.ascontiguousarray(np.asarray(v, dtype=np.float32)).reshape(B * H, S, D)

    qsq = (q.astype(np.float32) ** 2).sum(-1)  # [BH, S]
    ksq = (k.astype(np.float32) ** 2).sum(-1)

    # pt <= e^{0.125*max(qsq) + C}; keep under ~e^{10.5} (fp16 max 65504)
    C = float(min(10.5 - SM * qsq.max(), 0.0))
    qT = np.zeros((B * H, D + 2, S), np.float16)
    kT = np.zeros((B * H, D + 2, S), np.float16)
    qT[:, :D, :] = q.transpose(0, 2, 1)
    kT[:, :D, :] = k.transpose(0, 2, 1)
    qT[:, D, :] = -0.5
    qT[:, D + 1, :] = -0.5
    khi = ksq.astype(np.float16)
    klo = (ksq - khi.astype(np.float32)).astype(np.float16)
    kT[:, D, :] = khi
    kT[:, D + 1, :] = klo
    # V zero-padded to [S, 128]: PV stationary becomes [128,128] (M=128 like
    # every other matmul) so consecutive LDWEIGHTS overlap on the PE
    vin = np.zeros((B * H, S, 128), np.float16)
    vin[:, :, :D] = v.astype(np.float16)
    # host applies D_q * e^{-C}
    row_scale = np.exp(-SM * qsq.astype(np.float64) - C).astype(np.float32)

    # mbias[k_local, q_local] = 0 where q_local >= k_local (valid), else -60000
    r = np.arange(128)[:, None]
    c = np.arange(128)[None, :]
    mb = np.where(c >= r, 0.0, MASK_NEG)
    mbias = mb.astype(np.float16)
    mbias32 = mb.astype(np.float32)
    ident = np.eye(128, dtype=np.float16)
    biasc = np.full((128, 1), C, dtype=np.float32)

    in_maps = []
    for core in range(N_CORES):
        sl = slice(core * HPC, (core + 1) * HPC)
        in_maps.append(
            {
                "qT": np.ascontiguousarray(qT[sl]),
                "kT": np.ascontiguousarray(kT[sl]),
                "v": np.ascontiguousarray(vin[sl]),
                "mbias": mbias,
                "ident": ident,
                "mbias32": mbias32,
                "biasc": biasc,
            }
        )
    return in_maps, row_scale


def _gather(results, row_scale):
    """results[core]["ot"] : [HPC, D, S] -> full [B, H, S, D] (applies D_q)."""
    outs = [np.asarray(r["ot"]) for r in results]
    o = np.concatenate(outs, axis=0)  # [BH, D, S]
    o = o.transpose(0, 2, 1) * row_scale[:, :, None]  # [BH, S, D]
    return np.ascontiguousarray(o.reshape(B, H, S, D).astype(np.float32))


def kernel(q, k, v):
    from concourse.bass_utils import run_bass_kernel_spmd

    nc = _build_module()
    in_maps, row_scale = _host_prep(q, k, v)
    res = run_bass_kernel_spmd(nc, in_maps, core_ids=list(range(N_CORES)))
    return _gather(res.results, row_scale)


if __name__ == "__main__":
    rng = np.random.default_rng(0)
    q = rng.standard_normal((B, H, S, D), dtype=np.float32)
    k = rng.standard_normal((B, H, S, D), dtype=np.float32)
    v = rng.standard_normal((B, H, S, D), dtype=np.float32)
    o = kernel(q, k, v)
    print("out", o.shape, o.dtype, float(np.abs(o).max()))


# revision 19
# speedup vs baseline: 1.0721x; 1.0721x over previous
"""Trainium2 Bass kernel for causal RBF (squared-exponential) attention.

  p_ij = exp(-sm * ||q_i - k_j||^2) causal-masked, out = p @ v (no normalization)
  B,H,S,D = 2,16,2048,64 ; sm = 0.125

Sharding: B*H = 32 heads, 4 heads per core across 8 NeuronCores (head
parallel, no cross-core comm).

Factorization: p = D_q . exp(2*sm*qk - sm*||k||^2) with D_q diagonal in
exp(-sm*||q||^2). D_q is applied to the output rows on the host (O(N));
the device computes, per 128-key x 512-query tile, one fp16 matmul with a
66-deep contraction (rows 0..63 = K^T/Q^T, rows 64/65 = ksq split hi/lo
against -1/2 const rows), then ScalarE evaluates
    pt = exp(0.25 * psum + C)       (psum = qk - ksq/2)
straight PSUM -> SBUF. C is a host-supplied bias chosen from max(qsq) so
that pt <= e^{0.125 qsq + C} stays inside fp16 range (p <= 1 identity).

v2 scheduling insights (measured on this part):
  * The PE HAM clock gate only releases (1.2 -> 2.4 GHz) when the PE
    issue stream is gap-free for ~3.4us, and re-throttles on micro-idles.
  * LDWEIGHTS of consecutive matmuls overlap ONLY when the stationary
    free size (M = output partition width) is unchanged; M flips
    serialize LDW (+~112ns per MM).
  So v2 makes EVERY matmul M=128: V is zero-padded to [S,128] so the PV
  stationary is [128,128] (po rows 64..127 accumulate zeros); the causal
  mask is applied by an identity-stationary matmul accumulating a -60000
  fp16 bias tile into the diagonal corner (PE, not DVE); and the PE
  stream is kept dense with zero-stationary filler matmuls that
  accumulate exact zeros into the open PV bank (no extra PSUM, no deps).
PV: out^T[128, 512] += Vpad^T_kk @ P^T_kk accumulated in PSUM over kk;
rows 0..63 are the real output. out^T goes back in [D, S] layout; the
host transposes and applies D_q * e^{-C}. All O(S^2) work stays on
device.

Schedule notes: logits PSUM pool is GROUP=2 tiles x LG_BUFS=3 buffers
(6 banks) + 2 PV banks = 8. Output DMAs ride the gpsimd SWDGE queue so
the SP HWDGE queue never head-of-line blocks the next head's input
DMAs. Input DMAs are split per span so the first matmul starts after
~1/4 of the first transfer. A 1-element exp at t=0 pulls the ACT table
load off the critical path.
"""

import os
import sys

if "/opt/trn_rl_repo" not in sys.path:
    sys.path.insert(0, "/opt/trn_rl_repo")

import numpy as np

B, H, S, D = 2, 16, 2048, 64
SM = 0.125
N_CORES = 8
HPC = (B * H) // N_CORES  # heads per core = 4
SPAN = 512  # query-span per PSUM accumulation group
NSPAN = S // SPAN  # 4
KTILE = 128  # key rows per logits tile
EXP_SCALE = 2.0 * SM  # 0.25
MASK_NEG = -60000.0  # fp16-representable; 0.25*(-60000) -> exp underflows to 0

# knobs
GROUP = int(os.environ.get("KRN_GROUP", "2"))  # logits tiles per ACTIVATE
LG_BUFS = int(os.environ.get("KRN_LG_BUFS", "3"))  # logits psum buffers
WARMUP = int(os.environ.get("KRN_WARMUP", "8"))  # PE warmup matmuls
PT_BUFS = int(os.environ.get("KRN_PT_BUFS", "6"))
SPLIT_IN = os.environ.get("KRN_SPLIT_IN", "1") == "1"  # per-span input DMAs
ODMA_GP = os.environ.get("KRN_ODMA_GP", "1") == "1"  # out DMA on gpsimd queue
FILL_N = int(os.environ.get("KRN_FILL_N", "1"))  # zero-fill MMs per chunk
FILL_FREE = int(os.environ.get("KRN_FILL_FREE", "512"))  # zero-fill MM width
FILL_AUTO = os.environ.get("KRN_FILL_AUTO", "1") == "1"  # model-based fill size
FILL_ADJ = float(os.environ.get("KRN_FILL_ADJ", "0"))  # ns bias on fill budget
PE_MASK = os.environ.get("KRN_PE_MASK", "1") == "1"  # mask via PE (else DVE)
REORDER = os.environ.get("KRN_REORDER", "1") == "1"  # optimize chunk order

_CACHE = {}


def _build_module():
    """Build + compile the Bass module (once per process per variant)."""
    key = (GROUP, LG_BUFS, WARMUP, PT_BUFS, SPLIT_IN, ODMA_GP, FILL_N, FILL_FREE,
           PE_MASK, FILL_AUTO, FILL_ADJ, REORDER)
    if key in _CACHE:
        return _CACHE[key]

    import concourse.mybir as mybir
    import concourse.tile as tile
    from concourse import bacc

    f32 = mybir.dt.float32
    mmdt = mybir.dt.float16

    nc = bacc.Bacc(
        "TRN2", target_bir_lowering=False, debug=False, num_devices=N_CORES
    )

    qT = nc.dram_tensor("qT", [HPC, D + 2, S], mmdt, kind="ExternalInput").ap()
    kT = nc.dram_tensor("kT", [HPC, D + 2, S], mmdt, kind="ExternalInput").ap()
    v = nc.dram_tensor("v", [HPC, S, 128], mmdt, kind="ExternalInput").ap()
    mbias = nc.dram_tensor("mbias", [128, 128], mmdt, kind="ExternalInput").ap()
    ident = nc.dram_tensor("ident", [128, 128], mmdt, kind="ExternalInput").ap()
    mbias32 = nc.dram_tensor("mbias32", [128, 128], f32, kind="ExternalInput").ap()
    biasc = nc.dram_tensor("biasc", [128, 1], f32, kind="ExternalInput").ap()
    ot = nc.dram_tensor("ot", [HPC, D, S], f32, kind="ExternalOutput").ap()

    out_dma = nc.gpsimd.dma_start if ODMA_GP else nc.sync.dma_start

    with tile.TileContext(nc) as tc:
        with (
            tc.tile_pool(name="consts", bufs=1) as consts,
            tc.tile_pool(name="qk_sb", bufs=2) as qk_sb,
            tc.tile_pool(name="v_sb", bufs=2) as v_sb,
            tc.tile_pool(name="pt_sb", bufs=PT_BUFS) as pt_sb,
            tc.tile_pool(name="ot_sb", bufs=2) as ot_sb,
            tc.tile_pool(name="lg_ps", bufs=LG_BUFS, space="PSUM") as lg_ps,
            tc.tile_pool(name="pv_ps", bufs=2, space="PSUM") as pv_ps,
        ):
            # tiny exp at t=0: pulls the ~2.7us ACT table load off the
            # critical path (overlaps the first input DMAs + PE warmups)
            actw = consts.tile([1, 1], f32, tag="actw")
            nc.gpsimd.memset(actw, 0.0)
            actwo = consts.tile([1, 1], f32, tag="actwo")
            nc.scalar.activation(
                actwo, actw, mybir.ActivationFunctionType.Exp, bias=0.0, scale=1.0
            )

            # consts ride the Activation HWDGE queue (idle at startup) so the
            # SP queue dispatches kta0/qta0/v0 without delay
            biassb = consts.tile([128, 1], f32, tag="bias")
            nc.scalar.dma_start(out=biassb, in_=biasc)
            if PE_MASK:
                mbsb = consts.tile([128, 128], mmdt, tag="mbias")
                nc.scalar.dma_start(out=mbsb, in_=mbias)
                idsb = consts.tile([128, 128], mmdt, tag="ident")
                nc.scalar.dma_start(out=idsb, in_=ident)
            else:
                mbsb32 = consts.tile([128, 128], f32, tag="mbias32")
                nc.scalar.dma_start(out=mbsb32, in_=mbias32)

            # warmup / filler sources: no DMA dependency
            wsrc = consts.tile([128, SPAN], mmdt, tag="wsrc")
            nc.gpsimd.memset(wsrc, 1.0)
            wzero = consts.tile([128, 128], mmdt, tag="wzero")
            nc.gpsimd.memset(wzero, 0.0)

            # dense dummy matmuls at start: trip the PE HAM clock-gate and
            # bridge into the first real matmuls (which wait on DMA).
            for w in range(WARMUP):
                wps = pv_ps.tile([128, SPAN], f32, tag="po")
                nc.tensor.matmul(
                    wps, wsrc[:, 0:128], wsrc[:, 0:SPAN], start=True, stop=True
                )

            # --- static chunk geometry (identical for every head) ---
            # Used to size the zero-fill matmuls so the PE segment between
            # consecutive exp-ready points matches each exp's duration:
            #   fill(c-1) + PV(c-1) + L(c+1) + mask(c+1)  ~=  exp_ns(c)
            # (PE stream: L_c, mask_c, fill(c-1), PV(c-1), L_{c+1}, ...)
            # Chunks within a span may be processed in any order (PV
            # accumulation commutes); pick the order that minimizes the
            # unfillable ACT-starvation (negative-budget) spots.
            ndiag_ = SPAN // KTILE
            spans_geo = []
            for s_ in range(NSPAN):
                nkk_ = (s_ + 1) * ndiag_
                ch_ = []
                for g0 in range(0, nkk_, GROUP):
                    gkk_ = list(range(g0, min(g0 + GROUP, nkk_)))
                    n_ = len(gkk_)
                    gjd0_ = gkk_[0] - s_ * ndiag_
                    gc0_ = gjd0_ * KTILE if gjd0_ > 0 else 0
                    nmask_ = sum(1 for kk in gkk_ if kk - s_ * ndiag_ >= 0)
                    pv_cols_ = sum(
                        SPAN - (jd * KTILE if jd > 0 else 0)
                        for jd in (kk - s_ * ndiag_ for kk in gkk_)
                    )
                    ch_.append(
                        dict(
                            gkk=gkk_,
                            L_ns=n_ * ((SPAN - gc0_) / 2.4 + 2.5),
                            mask_ns=nmask_ * (KTILE / 2.4 + 2.5),
                            pv_ns=pv_cols_ / 2.4 + n_ * 2.5,
                            exp_ns=n_ * (SPAN - gc0_) * 0.8333 + 261.0,
                        )
                    )
                spans_geo.append(ch_)

            OVH = max(80.0, -FILL_ADJ)

            def _deficit(seq):
                N = len(seq)
                return sum(
                    max(
                        0.0,
                        seq[(i - 1) % N]["pv_ns"]
                        + seq[(i + 1) % N]["L_ns"]
                        + seq[(i + 1) % N]["mask_ns"]
                        + OVH
                        - seq[i]["exp_ns"],
                    )
                    for i in range(N)
                )

            if REORDER:
                import itertools as _it

                orders = [list(range(len(sp))) for sp in spans_geo]

                def _build(o):
                    return [spans_geo[s][i] for s in range(NSPAN) for i in o[s]]

                cur = _deficit(_build(orders))
                improved = True
                rounds = 0
                while improved and rounds < 12:
                    improved = False
                    rounds += 1
                    for s_ in range(NSPAN):
                        nsp = len(spans_geo[s_])
                        cand = (
                            _it.permutations(range(nsp))
                            if nsp <= 6
                            else ([r % nsp for r in range(k, k + nsp)] for k in range(nsp))
                        )
                        for p in cand:
                            o2 = [list(o) for o in orders]
                            o2[s_] = list(p)
                            d_ = _deficit(_build(o2))
                            if d_ < cur - 1e-9:
                                cur = d_
                                orders = o2
                                improved = True
            else:
                orders = [list(range(len(sp))) for sp in spans_geo]

            sched = []  # flat ordered chunk schedule (per head)
            span_chunks = []  # per span: ordered list of chunk dicts
            for s_ in range(NSPAN):
                osp = [spans_geo[s_][i] for i in orders[s_]]
                span_chunks.append(osp)
                sched.extend(osp)
            geo = sched
            NCH = len(geo)

            def fill_cols_for(ci):
                # pending chunk index ci is PV'd during chunk ci+1; size the
                # fill so ACT never starves at exp(ci+1) (cyclic across heads)
                if not FILL_AUTO:
                    return [FILL_FREE] * FILL_N
                budget = (
                    geo[(ci + 1) % NCH]["exp_ns"]
                    - geo[ci]["pv_ns"]
                    - geo[(ci + 2) % NCH]["L_ns"]
                    - geo[(ci + 2) % NCH]["mask_ns"]
                    + FILL_ADJ
                )
                cols = int(budget * 2.4 // 64) * 64
                out = []
                while cols >= 64:
                    c = min(cols, SPAN)
                    out.append(c)
                    cols -= c
                return out

            def emit_pv(pend, tail=False):
                # PV matmuls for a completed exp group (trails the logits of
                # the next group so the in-order PE stream never stalls on
                # ACT). Returns a deferred span-finalize closure, or None.
                po_, pt_, gkk_, s_, vsb_, h_, ci_, firstc_, lastc_ = pend
                ndiag = SPAN // KTILE
                # zero-stationary fillers: accumulate exact zeros into the
                # open PV bank. They have no input deps, so the PE chews on
                # them while ACT finishes the exp this group's PV waits on —
                # keeps the issue stream gap-free (HAM stays at K=8/8).
                for fc in [] if tail else fill_cols_for(ci_):
                    nc.tensor.matmul(
                        po_[:, 0:fc],
                        wzero,
                        wsrc[:, 0:fc],
                        start=False,
                        stop=False,
                        skip_group_check=True,
                    )
                n_ = len(gkk_)
                for j, kk in enumerate(gkk_):
                    jd = kk - s_ * ndiag
                    # positional flags: chunks may be reordered within a span
                    first = firstc_ and j == 0
                    last = lastc_ and j == n_ - 1
                    c0 = jd * KTILE if jd > 0 else 0
                    nc.tensor.matmul(
                        po_[:, c0:SPAN],
                        vsb_[:, kk, :],
                        pt_[:, j, c0:SPAN],
                        start=first,
                        stop=last,
                        skip_group_check=True,
                    )
                if not lastc_:
                    return None

                def finalize():
                    # span finished: copy PSUM->SBUF and DMA out. Emitted one
                    # chunk late so the copy never head-of-line blocks the
                    # next chunk's work on the in-order DVE queue.
                    oT = ot_sb.tile([D, SPAN], f32, tag="oT")
                    nc.vector.tensor_copy(oT, po_[0:D, :])
                    # very last transfer takes the low-latency SP HWDGE path
                    # (queue is empty by then); the rest stay off SP so they
                    # never head-of-line block the next head's input DMAs
                    last = h_ == HPC - 1 and s_ == NSPAN - 1
                    dma = nc.sync.dma_start if last else out_dma
                    dma(out=ot[h_, :, s_ * SPAN : (s_ + 1) * SPAN], in_=oT)

                return finalize

            pending = None
            pending_out = None

            for h in range(HPC):
                qta = qk_sb.tile([D + 2, S], mmdt, tag="qta")
                kta = qk_sb.tile([D + 2, S], mmdt, tag="kta")
                vsb = v_sb.tile([128, S // 128, 128], mmdt, tag="vsb")
                vr = v[h].rearrange("(t p) d -> p t d", p=128)
                if SPLIT_IN and h == 0:
                    # head 0 only: per-span pieces so span 0's gate
                    # (kta0+qta0) lands after 2 transfers; later heads
                    # prefetch behind compute so whole-tensor DMAs are
                    # cheaper on the HWDGE queue (fixed per-DMA cost)
                    for s in range(NSPAN):
                        sl = slice(s * SPAN, (s + 1) * SPAN)
                        nc.sync.dma_start(out=kta[:, sl], in_=kT[h][:, sl])
                        # span-0 qta rides the idle Pool SWDGE queue so
                        # kta0/qta0 dispatch in parallel (SP sequencer
                        # dispatch is ~650ns serial per DMA)
                        qdma = nc.gpsimd.dma_start if s == 0 else nc.sync.dma_start
                        qdma(out=qta[:, sl], in_=qT[h][:, sl])
                        tl = slice(
                            s * (S // 128 // NSPAN), (s + 1) * (S // 128 // NSPAN)
                        )
                        nc.sync.dma_start(out=vsb[:, tl, :], in_=vr[:, tl, :])
                else:
                    nc.sync.dma_start(out=qta, in_=qT[h])
                    nc.sync.dma_start(out=kta, in_=kT[h])
                    nc.sync.dma_start(out=vsb, in_=vr)

                ci = -1
                for s in range(NSPAN):
                    ndiag = SPAN // KTILE
                    po = pv_ps.tile([128, SPAN], f32, tag="po")
                    qspan = qta[:, s * SPAN : (s + 1) * SPAN]
                    ordered = span_chunks[s]
                    for oi, chd in enumerate(ordered):
                        ci += 1
                        gkk = chd["gkk"]
                        firstc = oi == 0
                        lastc = oi == len(ordered) - 1
                        n = len(gkk)
                        pl = lg_ps.tile([128, GROUP, SPAN], f32, tag="pl")
                        gjd0 = gkk[0] - s * ndiag
                        gc0 = gjd0 * KTILE if gjd0 > 0 else 0
                        diag_js = [
                            (j, kk - s * ndiag)
                            for j, kk in enumerate(gkk)
                            if kk - s * ndiag >= 0
                        ]
                        for j, kk in enumerate(gkk):
                            jd = kk - s * ndiag
                            # trim only to the group-common dead prefix so
                            # the grouped exp below reads fully-written PSUM
                            nc.tensor.matmul(
                                pl[:, j, gc0:SPAN],
                                kta[:, kk * KTILE : (kk + 1) * KTILE],
                                qspan[:, gc0:SPAN],
                                start=True,
                                stop=not (PE_MASK and jd >= 0),
                            )
                        if PE_MASK:
                            # causal mask via PE: identity-stationary matmul
                            # accumulates the -60000 corner into the diagonal
                            # 128x128 block (keeps the mask off the DVE and
                            # the stationary M=128 so LDWEIGHTS stays hidden)
                            for j, jd in diag_js:
                                nc.tensor.matmul(
                                    pl[:, j, jd * KTILE : (jd + 1) * KTILE],
                                    idsb,
                                    mbsb,
                                    start=False,
                                    stop=True,
                                    skip_group_check=True,
                                )
                        else:
                            for j, jd in diag_js:
                                nc.vector.tensor_add(
                                    pl[:, j, jd * KTILE : (jd + 1) * KTILE],
                                    pl[:, j, jd * KTILE : (jd + 1) * KTILE],
                                    mbsb32,
                                )
                        if pending is not None:
                            fin = emit_pv(pending)
                            pending = None
                            if pending_out is not None:
                                pending_out()
                            pending_out = fin
                        pt = pt_sb.tile([128, GROUP, SPAN], mmdt, tag="pt")
                        nc.scalar.activation(
                            pt[:, 0:n, gc0:SPAN],
                            pl[:, 0:n, gc0:SPAN],
                            mybir.ActivationFunctionType.Exp,
                            bias=biassb,
                            scale=EXP_SCALE,
                        )
                        pending = (po, pt, gkk, s, vsb, h, ci, firstc, lastc)
            if pending is not None:
                fin = emit_pv(pending, tail=True)
                pending = None
                if pending_out is not None:
                    pending_out()
                if fin is not None:
                    fin()
                pending_out = None

    nc.compile()
    _CACHE[key] = nc
    return nc


def _host_prep(q, k, v):
    """Shard + relayout inputs for the 8 cores. Returns (in_maps, row_scale)."""
    q = np.ascontiguousarray(np.asarray(q, dtype=np.float32)).reshape(B * H, S, D)
    k = np.ascontiguousarray(np.asarray(k, dtype=np.float32)).reshape(B * H, S, D)
    v = np.ascontiguousarray(np.asarray(v, dtype=np.float32)).reshape(B * H, S, D)

    qsq = (q.astype(np.float32) ** 2).sum(-1)  # [BH, S]
    ksq = (k.astype(np.float32) ** 2).sum(-1)

    # pt <= e^{0.125*max(qsq) + C}; keep under ~e^{10.5} (fp16 max 65504)
    C = float(min(10.5 - SM * qsq.max(), 0.0))
    qT = np.zeros((B * H, D + 2, S), np.float16)
    kT = np.zeros((B * H, D + 2, S), np.float16)
    qT[:, :D, :] = q.transpose(0, 2, 1)
    kT[:, :D, :] = k.transpose(0, 2, 1)
    qT[:, D, :] = -0.5
    qT[:, D + 1, :] = -0.5
    khi = ksq.astype(np.float16)
    klo = (ksq - khi.astype(np.float32)).astype(np.float16)
    kT[:, D, :] = khi
    kT[:, D + 1, :] = klo
    # V zero-padded to [S, 128]: PV stationary becomes [128,128] (M=128 like
    # every other matmul) so consecutive LDWEIGHTS overlap on the PE
    vin = np.zeros((B * H, S, 128), np.float16)
    vin[:, :, :D] = v.astype(np.float16)
    # host applies D_q * e^{-C}
    row_scale = np.exp(-SM * qsq.astype(np.float64) - C).astype(np.float32)

    # mbias[k_local, q_local] = 0 where q_local >= k_local (valid), else -60000
    r = np.arange(128)[:, None]
    c = np.arange(128)[None, :]
    mb = np.where(c >= r, 0.0, MASK_NEG)
    mbias = mb.astype(np.float16)
    mbias32 = mb.astype(np.float32)
    ident = np.eye(128, dtype=np.float16)
    biasc = np.full((128, 1), C, dtype=np.float32)

    in_maps = []
    for core in range(N_CORES):
        sl = slice(core * HPC, (core + 1) * HPC)
        in_maps.append(
            {
                "qT": np.ascontiguousarray(qT[sl]),
                "kT": np.ascontiguousarray(kT[sl]),
                "v": np.ascontiguousarray(vin[sl]),
                "mbias": mbias,
                "ident": ident,
                "mbias32": mbias32,
                "biasc": biasc,
            }
        )
    return in_maps, row_scale


def _gather(results, row_scale):
    """results[core]["ot"] : [HPC, D, S] -> full [B, H, S, D] (applies D_q)."""
    outs = [np.asarray(r["ot"]) for r in results]
    o = np.concatenate(outs, axis=0)  # [BH, D, S]
    o = o.transpose(0, 2, 1) * row_scale[:, :, None]  # [BH, S, D]
    return np.ascontiguousarray(o.reshape(B, H, S, D).astype(np.float32))


def kernel(q, k, v):
    from concourse.bass_utils import run_bass_kernel_spmd

    nc = _build_module()
    in_maps, row_scale = _host_prep(q, k, v)
    res = run_bass_kernel_spmd(nc, in_maps, core_ids=list(range(N_CORES)))
    return _gather(res.results, row_scale)


if __name__ == "__main__":
    rng = np.random.default_rng(0)
    q = rng.standard_normal((B, H, S, D), dtype=np.float32)
    k = rng.standard_normal((B, H, S, D), dtype=np.float32)
    v = rng.standard_normal((B, H, S, D), dtype=np.float32)
    o = kernel(q, k, v)
    print("out", o.shape, o.dtype, float(np.abs(o).max()))
